# revision 38
# baseline (speedup 1.0000x reference)
"""AdaptiveRankSemiseparableLayer on 8 trn2 NeuronCores.

Reference semantics (B=4, L=4096, D=1024, R=32, GH=256):
    h     = relu(x @ gate_w1 + gate_b1)            # (B,L,GH)
    gate  = sigmoid(h @ gate_w2 + gate_b2)         # (B,L,R)
    U     = x @ U_w ;  V = x @ V_w                 # (B,L,R)
    S     = cumsum(V, axis=1)                      # causal scan
    y_g   = (gate*U*S) @ out_w + out_b             # (B,L,D)
    t_out = depthwise_conv1d(x, conv_w, k=3, pad 1)
    out   = t_out + y_g

Sharding: 8 shards of 2048 contiguous tokens (2 per batch); host-side
bf16 pre-cast, stored transposed ([D, tokens+halo]) so the device needs
only plain wide DMAs.

The cumsum carry for the second half of each batch is RECOMPUTED from
the neighbor's raw tokens instead of communicated (a collective cannot
complete before this runtime's ~55us entry barrier):
    carry = (sum_t x_neighbor[t]) @ V_w
via one extra 4MB input (xn; zeros on even cores), 8 parallel
DMA-accumulate chains (CCE adds in the DMA datapath, no engine time),
and a handful of tiny matmuls.

Device layout: transposed everywhere (d / gh / r on partitions, tokens
on the free dim).  Conv: x[t-1] tap on ACT (Identity: w0*x + out_b),
x[t+1] tap on DVE STT (4-byte-aligned 2-element shift, in-place),
center tap as diagonal matmuls accumulated with the R->D projection of
glob=(S+carry)*gate*U.  The output matmuls are interleaved with the
gate MLP per token group so the PE never idles (its HAM clock gate
would halve the clock after ~3.4us of idle); dummy matmuls warm it at
kernel start.
"""

import numpy as np
import ml_dtypes

from concourse import bacc, mybir, tile
from concourse.bass_utils import run_bass_kernel_spmd

F32 = mybir.dt.float32
BF16 = mybir.dt.bfloat16
AX = mybir.AluOpType
AF = mybir.ActivationFunctionType
BF16NP = ml_dtypes.bfloat16

B, L, D, R, GH = 4, 4096, 1024, 32, 256
NCORES = 8
TOK = 2048          # tokens per core
G = 512             # token group (matmul rhs free size)
NG = TOK // G       # 4 groups
NCH = D // 128      # 8 d-chunks
XROWS = 2064        # 1 halo + 2048 + 1 halo + pad
NWARM = 10          # HAM warm-up matmuls
NACC = 8            # parallel xn accumulate chains


def _build(weights_np):
    nc = bacc.Bacc(None, target_bir_lowering=False, debug=False)

    x_ext = nc.declare_dram_parameter("x", [D, XROWS], BF16, isOutput=False)
    xn_ext = nc.declare_dram_parameter("xn", [TOK, D], BF16, isOutput=False)
    y_ext = nc.declare_dram_parameter("y", [NG, 128, NCH, G], BF16, isOutput=True)

    cw = {k: nc.inline_tensor(v, name=k) for k, v in weights_np.items()}

    with tile.TileContext(nc) as tc:
        with (
            tc.tile_pool(name="wsb", bufs=1) as wsb,
            tc.tile_pool(name="xsb", bufs=1) as xsb,
            tc.tile_pool(name="hsb", bufs=3) as hsb,
            tc.tile_pool(name="ssb", bufs=2) as ssb,
            tc.tile_pool(name="ysb", bufs=2) as ysb,
            tc.tile_pool(name="uvps", bufs=1, space="PSUM") as uvps,
            tc.tile_pool(name="vtps", bufs=1, space="PSUM") as vtps,
            tc.tile_pool(name="hps", bufs=1, space="PSUM") as hps,
            tc.tile_pool(name="gps", bufs=1, space="PSUM") as gps,
            tc.tile_pool(name="yps", bufs=4, space="PSUM") as yps,
        ):
            # ---- first weights, then whole-chunk x loads on both queues ----
            uvwsb = wsb.tile([128, NCH * 2 * R], BF16, name="uvwsb")
            nc.sync.dma_start(out=uvwsb[:, :], in_=cw["uvw"][:, :])
            smallsb = wsb.tile([128, 28], F32, name="smallsb")
            nc.sync.dma_start(out=smallsb[:, :], in_=cw["small"][:, :])
            # small cols: 0:8 w0col, 8:16 out_b, 16:24 w2col, 24:26 b1, 26 b2
            w0col = smallsb[:, 0:8]
            outb = smallsb[:, 8:16]
            w2col = smallsb[:, 16:24]
            b1 = smallsb[:, 24:26]
            b2 = smallsb[0:R, 26:27]

            xT = [xsb.tile([128, XROWS], BF16, name=f"xT{c}") for c in range(NCH)]
            w1sb = wsb.tile([128, NCH * GH], BF16, name="w1sb")
            w2sb = wsb.tile([128, 2 * R], BF16, name="w2sb")
            outwsb = wsb.tile([R, D], BF16, name="outwsb")
            diag1sb = wsb.tile([128, NCH * 128], BF16, name="diag1sb")
            diag2sb = wsb.tile([128, NCH * 128], BF16, name="diag2sb")
            for c in [0, 2, 4, 6]:
                nc.sync.dma_start(out=xT[c][:, :], in_=x_ext[c * 128:(c + 1) * 128, :])
            nc.sync.dma_start(out=w2sb[:, :], in_=cw["w2"][:, :])
            nc.sync.dma_start(out=outwsb[:, :], in_=cw["outw"][:, :])
            for c in [1, 3]:
                nc.scalar.dma_start(out=xT[c][:, :], in_=x_ext[c * 128:(c + 1) * 128, :])
            nc.scalar.dma_start(out=w1sb[:, :], in_=cw["w1"][:, :])
            for c in [5, 7]:
                nc.scalar.dma_start(out=xT[c][:, :], in_=x_ext[c * 128:(c + 1) * 128, :])
            nc.scalar.dma_start(out=diag1sb[:, :], in_=cw["diag1"][:, :])
            nc.sync.dma_start(out=diag2sb[:, :], in_=cw["diag2"][:, :])

            # ---- neighbor tokens: plain loads on the HWDGE queues, right
            #      after x so they never contend with the critical loads ----
            xaccs = [wsb.tile([128, D], BF16, name=f"xacc{a}") for a in range(16)]
            for i, t in enumerate([x for k in range(8) for x in (k, k + 8)]):
                eng = nc.sync if i % 2 == 0 else nc.scalar
                eng.dma_start(out=xaccs[t][:, :], in_=xn_ext[t * 128:(t + 1) * 128, :])

            # ---- conv tap x[t-1] on ACT: t_sb = w0*x + out_b (emitted in
            #      slices interleaved with the relu/sigmoid stream) ----
            t_sb = xsb.tile([128, NCH * TOK], BF16, name="t_sb")

            def emit_tap1(cs):
                for c in cs:
                    nc.scalar.activation(
                        t_sb[:, c * TOK:(c + 1) * TOK], xT[c][:, 0:TOK],
                        AF.Identity, bias=outb[:, c:c + 1], scale=w0col[:, c:c + 1],
                    )

            # chunks 0-3 on DVE (4x tensor_scalar, fills its early idle window)
            for c in [0, 1, 2, 3]:
                nc.vector.tensor_scalar(
                    t_sb[:, c * TOK:(c + 1) * TOK], xT[c][:, 0:TOK],
                    w0col[:, c:c + 1], outb[:, c:c + 1], AX.mult, AX.add,
                )

            S_sb = ssb.tile([R, TOK], F32, name="S_sb", bufs=1)
            junk = ssb.tile([R, 1], F32, name="junk", bufs=1)
            nc.vector.memset(junk[:, :], 0.0)
            ones128 = wsb.tile([128, 1], BF16, name="ones128")
            nc.vector.memset(ones128[:, :], 1.0)

            # ---- HAM warm-up on the first-arriving weight tile ----
            for i in range(NWARM):
                warm = uvps.tile([2 * R, G], F32, name="uv")
                nc.tensor.matmul(
                    warm[:, :], uvwsb[:, 0:64], uvwsb[:, 0:G], start=True, stop=True
                )

            # ---- U,V projections; V scan chain runs on SBUF copies ----
            uv_sbs = []
            for g in range(NG):
                lo = 1 + g * G
                uv = uvps.tile([2 * R, G], F32, name="uv")
                for c in range(NCH):
                    nc.tensor.matmul(
                        uv[:, :], uvwsb[:, c * 64:(c + 1) * 64],
                        xT[c][:, lo:lo + G], start=(c == 0), stop=(c == NCH - 1),
                    )
                uvsb = ssb.tile([R, G], BF16, name="uvsb", bufs=4)
                nc.vector.tensor_copy(uvsb[:, :], uv[0:R, :])
                uv_sbs.append(uvsb)
                for k in range(2 * g, 2 * g + 2):
                    nc.vector.tensor_tensor(
                        xaccs[k][:, :], xaccs[k][:, :], xaccs[k + 8][:, :], AX.add
                    )
                if g == 3:
                    for k in (0, 1):
                        nc.vector.tensor_tensor(
                            xaccs[k][:, :], xaccs[k][:, :], xaccs[k + 4][:, :], AX.add
                        )
                # scan insts have no sync-wait slots; the uvsb copy above is the
                # same-engine touch that absorbs the PE dependency.
                nc.vector.tensor_tensor_scan(
                    S_sb[:, g * G:(g + 1) * G], uv[R:2 * R, :],
                    junk[:, 0:1].broadcast_to((R, G)),
                    0.0 if g == 0 else S_sb[:, g * G - 1:g * G], AX.add, AX.bypass,
                )

            # ---- carry = (sum_t xn[t]) @ V_w ----
            # remaining add-tree levels (levels 1 and half of 2 ran inside
            # the scan loop)
            for k in (2, 3):
                nc.vector.tensor_tensor(
                    xaccs[k][:, :], xaccs[k][:, :], xaccs[k + 4][:, :], AX.add
                )
            for step in (2, 1):
                for k in range(step):
                    nc.vector.tensor_tensor(
                        xaccs[k][:, :], xaccs[k][:, :], xaccs[k + step][:, :], AX.add
                    )
            xsums = vtps.tile([128, NCH], F32, name="vt", padded_shape=[128, 512])
            for c in range(NCH):
                nc.tensor.matmul(
                    xsums[:, c:c + 1], xaccs[0][:, c * 128:(c + 1) * 128],
                    ones128[:, 0:1], start=True, stop=True,
                )
            xsbf = ssb.tile([128, NCH], BF16, name="xsbf", bufs=1)
            nc.vector.tensor_copy(xsbf[:, :], xsums[:, :])
            carry_ps = vtps.tile([R, 1], F32, name="vt", padded_shape=[128, 512])
            for c in range(NCH):
                nc.tensor.matmul(
                    carry_ps[:, :], uvwsb[:, c * 64 + R:(c + 1) * 64],
                    xsbf[:, c:c + 1], start=(c == 0), stop=(c == NCH - 1),
                )
            carry = wsb.tile([R, 1], F32, name="carry")
            nc.vector.tensor_copy(carry[:, :], carry_ps[:, :])

            # ---- gate MLP interleaved with output matmuls (2-group lag
            #      keeps the in-order PE stream dense) ----
            def emit_mlp(g):
                lo = 1 + g * G
                htg = []
                for j in range(2):
                    hp = hps.tile([128, G], F32, name="hp")
                    for c in range(NCH):
                        nc.tensor.matmul(
                            hp[:, :],
                            w1sb[:, (c * 2 + j) * 128:(c * 2 + j + 1) * 128],
                            xT[c][:, lo:lo + G],
                            start=(c == 0), stop=(c == NCH - 1),
                        )
                    hs = hsb.tile([128, G], BF16, name="hs")
                    nc.scalar.activation(
                        hs[:, :], hp[:, :], AF.Relu, bias=b1[:, j:j + 1], scale=1.0
                    )
                    htg.append(hs)
                gp = gps.tile([R, G], F32, name="gp")
                for j in range(2):
                    nc.tensor.matmul(
                        gp[:, :], w2sb[:, j * R:(j + 1) * R], htg[j][:, :],
                        start=(j == 0), stop=(j == 1),
                    )
                gate = ssb.tile([R, G], BF16, name="gate", bufs=4)
                nc.scalar.activation(
                    gate[:, :], gp[:, :], AF.Sigmoid, bias=b2, scale=1.0
                )
                return gate

            def emit_glob(g, gate):
                t1 = hsb.tile([R, G], BF16, name="t1", bufs=4)
                nc.vector.tensor_mul(t1[:, :], gate[:, :], uv_sbs[g][:, :])
                glob = hsb.tile([R, G], BF16, name="glob", bufs=4)
                nc.vector.scalar_tensor_tensor(
                    glob[:, :], S_sb[:, g * G:(g + 1) * G], carry[:, 0:1],
                    t1[:, :], AX.add, AX.mult,
                )
                return glob

            def emit_out(g, glob):
                # software-pipelined at the PSUM slot depth: the two
                # glob-independent diag taps of tiles c+1, c+2 are emitted
                # before tile c's glob-gated projection, so the in-order PE
                # has dense work while the glob handoff resolves.
                lo = 1 + g * G
                y_sb = ysb.tile([128, NCH * G], BF16, name="y_sb")
                yps_tiles = []

                def diags(c):
                    yp = yps.tile([128, G], F32, name="yp")
                    nc.tensor.matmul(
                        yp[:, :], diag1sb[:, c * 128:(c + 1) * 128],
                        xT[c][:, lo:lo + G], start=True, stop=False,
                    )
                    nc.tensor.matmul(
                        yp[:, :], diag2sb[:, c * 128:(c + 1) * 128],
                        xT[c][:, lo + 1:lo + G + 1], start=False, stop=False,
                    )
                    yps_tiles.append(yp)

                def finish(c):
                    yp = yps_tiles[c]
                    nc.tensor.matmul(
                        yp[:, :], outwsb[:, c * 128:(c + 1) * 128], glob[:, :],
                        start=False, stop=True,
                    )
                    nc.vector.tensor_tensor(
                        y_sb[:, c * G:(c + 1) * G],
                        t_sb[:, c * TOK + g * G:c * TOK + (g + 1) * G],
                        yp[:, :], AX.add,
                    )
                    if c % 2 == 1:
                        eng = nc.sync if (g * 4 + c // 2) % 2 == 0 else nc.scalar
                        eng.dma_start(
                            out=y_ext[g, :, c - 1:c + 1, :],
                            in_=y_sb[:, (c - 1) * G:(c + 1) * G],
                        )

                diags(0)
                diags(1)
                for c in range(NCH):
                    if c + 2 < NCH:
                        diags(c + 2)
                    finish(c)

            gate0 = emit_mlp(0)
            glob0 = emit_glob(0, gate0)
            emit_tap1([4, 5])
            gate1 = emit_mlp(1)
            glob1 = emit_glob(1, gate1)
            emit_tap1([6, 7])
            emit_out(0, glob0)
            gate2 = emit_mlp(2)
            glob2 = emit_glob(2, gate2)
            emit_out(1, glob1)
            gate3 = emit_mlp(3)
            glob3 = emit_glob(3, gate3)
            emit_out(2, glob2)
            emit_out(3, glob3)

    nc.finalize()
    return nc


def _prep_weights(gate_w1, gate_b1, gate_w2, gate_b2, U_w, V_w, conv_w, out_w, out_b):
    bf = lambda a: np.ascontiguousarray(a).astype(BF16NP)
    f32 = lambda a: np.ascontiguousarray(a).astype(np.float32)
    w1 = np.concatenate([gate_w1[c * 128:(c + 1) * 128, :] for c in range(NCH)], axis=1)
    uvw = np.concatenate(
        [np.concatenate([U_w[c * 128:(c + 1) * 128, :], V_w[c * 128:(c + 1) * 128, :]], axis=1)
         for c in range(NCH)], axis=1)
    w2 = np.concatenate([gate_w2[j * 128:(j + 1) * 128, :] for j in range(2)], axis=1)
    diag1 = np.concatenate(
        [np.diag(conv_w[c * 128:(c + 1) * 128, 1]) for c in range(NCH)], axis=1)
    diag2 = np.concatenate(
        [np.diag(conv_w[c * 128:(c + 1) * 128, 2]) for c in range(NCH)], axis=1)
    small = np.zeros((128, 28), np.float32)
    small[:, 0:8] = conv_w[:, 0].reshape(NCH, 128).T
    small[:, 8:16] = out_b.reshape(NCH, 128).T
    small[:, 16:24] = conv_w[:, 2].reshape(NCH, 128).T
    small[:, 24:26] = gate_b1.reshape(2, 128).T
    small[0:R, 26] = gate_b2
    return {
        "w1": bf(w1), "uvw": bf(uvw), "w2": bf(w2), "outw": bf(out_w),
        "diag1": bf(diag1), "diag2": bf(diag2), "small": f32(small),
    }


def _shard_x(x):
    """Per-core transposed bf16 shard [D, XROWS] + neighbor tokens [TOK, D]."""
    shards, neighbors = [], []
    zeros = np.zeros((TOK, D), BF16NP)
    for c in range(NCORES):
        b, h = c // 2, c % 2
        t0 = h * TOK
        s = np.zeros((XROWS, D), np.float32)
        lo, hi = t0 - 1, t0 + TOK + 1
        src_lo, src_hi = max(lo, 0), min(hi, L)
        s[src_lo - lo:src_lo - lo + (src_hi - src_lo), :] = x[b, src_lo:src_hi, :]
        shards.append(np.ascontiguousarray(s.T).astype(BF16NP))
        if h == 1:
            neighbors.append(np.ascontiguousarray(x[b, 0:TOK, :]).astype(BF16NP))
        else:
            neighbors.append(zeros)
    return shards, neighbors


def _run(inputs, trace=False, tmpdir=None):
    x = np.asarray(inputs["x"], np.float32)
    weights = _prep_weights(
        *[np.asarray(inputs[k], np.float32) for k in
          ("gate_w1", "gate_b1", "gate_w2", "gate_b2", "U_w", "V_w",
           "conv_w", "out_w", "out_b")])
    nc = _build(weights)
    shards, neighbors = _shard_x(x)
    in_maps = [{"x": shards[c], "xn": neighbors[c]} for c in range(NCORES)]
    res = run_bass_kernel_spmd(
        nc, in_maps, core_ids=list(range(NCORES)), trace=trace, tmpdir=tmpdir
    )
    out = np.empty((B, L, D), np.float32)
    for c in range(NCORES):
        b, h = c // 2, c % 2
        yc = np.asarray(res.results[c]["y"]).astype(np.float32)
        # [g, p, ch, t] -> [(g t), (ch p)]
        yc = yc.transpose(0, 3, 2, 1).reshape(TOK, D)
        out[b, h * TOK:(h + 1) * TOK, :] = yc
    return out, res


def kernel(**inputs) -> np.ndarray:
    out, _ = _run(inputs)
    return out
